# revision 2
# baseline (speedup 1.0000x reference)
"""Trainium2 Bass kernel for nn_ABNet_U (multi-branch MLP + CBF-QP head).

Data-parallel over batch: 16384 rows -> 8 NeuronCores x 2048 rows.
Weights replicated, host-prepped into K-major bf16 layouts; all GEMMs run
on the TensorEngine with fp32 PSUM accumulation and fused bias+activation
eviction on the ScalarEngine.  The trig/QP tail runs in fp32 on the
Vector/Scalar engines with batch on partitions.
"""

import sys

sys.path.insert(0, "/opt/trn_rl_repo")

import numpy as np
import ml_dtypes

import concourse.bass as bass
import concourse.mybir as mybir
import concourse.tile as tile
from concourse import bacc
from concourse.bass_utils import run_bass_kernel_spmd
from concourse.masks import make_identity

N_CORES = 8
B_GLOBAL = 16384
B = B_GLOBAL // N_CORES  # 2048 rows per core
P = 128
CH = B // P              # 16 batch chunks of 128 (tail layout)
NF = 512                 # matmul free-dim chunk
NB = B // NF             # 4 free chunks
HEADS = 10

AF = mybir.ActivationFunctionType
ALU = mybir.AluOpType
AX = mybir.AxisListType
F32 = mybir.dt.float32
BF16 = mybir.dt.bfloat16
I32 = mybir.dt.int32

TWO_PI = float(2.0 * np.pi)
HALF_PI = float(0.5 * np.pi)

_CACHED_NC = None


def _build():
    nc = bacc.Bacc(
        "TRN2",
        target_bir_lowering=False,
        debug=False,
        enable_asserts=False,
        num_devices=N_CORES,
    )

    def din(name, shape, dt=F32):
        return nc.dram_tensor(name, list(shape), dt, kind="ExternalInput").ap()

    xt = din("xt", (4, B), BF16)            # x shard, transposed, bf16
    xn = din("xn", (P, CH, 4))              # x shard, [p, chunk, feat] fp32
    w1 = din("w1", (4, 2048), BF16)
    w2 = din("w2", (P, 16, 16, P), BF16)    # [p, mt, kt, mc]
    w3 = din("w3", (P, 16, 16, P), BF16)
    w41 = din("w41", (P, 8, 8, P), BF16)
    w42 = din("w42", (P, 8, 8, P), BF16)
    w51 = din("w51", (P, 8, 20), BF16)      # [p, kt, m]
    w52 = din("w52", (P, 8, 11), BF16)
    b1 = din("b1", (P, 16))
    b2 = din("b2", (P, 16))
    b3 = din("b3", (P, 16))
    b41 = din("b41", (P, 8))
    b42 = din("b42", (P, 8))
    b51 = din("b51", (20,))
    b52 = din("b52", (11,))
    stdb = din("stdb", (P, 4))
    meanb = din("meanb", (P, 4))
    mlb = din("mlb", (P, 2))
    islb = din("islb", (P, 2))
    wtv = din("wtv", (10,))
    out = nc.dram_tensor("out", [P, CH, 2], F32, kind="ExternalOutput").ap()

    with tile.TileContext(nc) as tc:
        from contextlib import ExitStack

        with ExitStack() as ctx:
            const = ctx.enter_context(tc.tile_pool(name="const", bufs=1))
            wpool = ctx.enter_context(tc.tile_pool(name="wpool", bufs=3))
            hpool = ctx.enter_context(tc.tile_pool(name="hpool", bufs=2))
            psum = ctx.enter_context(tc.tile_pool(name="psum", bufs=4, space="PSUM"))
            pstr = ctx.enter_context(tc.tile_pool(name="pstr", bufs=2, space="PSUM"))
            tp = ctx.enter_context(tc.tile_pool(name="tp", bufs=1))

            # ---- constants ----
            ident = const.tile([P, P], F32)
            make_identity(nc, ident[:])
            halfpi = const.tile([P, 1], F32, tag="halfpi")
            nc.vector.memset(halfpi[:], HALF_PI)

            b1t = const.tile([P, 16], F32, tag="b1")
            nc.sync.dma_start(b1t[:], b1)
            b2t = const.tile([P, 16], F32, tag="b2")
            nc.sync.dma_start(b2t[:], b2)
            b3t = const.tile([P, 16], F32, tag="b3")
            nc.sync.dma_start(b3t[:], b3)
            b41t = const.tile([P, 8], F32, tag="b41")
            nc.sync.dma_start(b41t[:], b41)
            b42t = const.tile([P, 8], F32, tag="b42")
            nc.sync.dma_start(b42t[:], b42)
            b51t = const.tile([20, 1], F32, tag="b51")
            nc.sync.dma_start(b51t[:], b51[:, None])
            b52t = const.tile([11, 1], F32, tag="b52")
            nc.sync.dma_start(b52t[:], b52[:, None])
            stdt = const.tile([P, 4], F32, tag="stdt")
            nc.sync.dma_start(stdt[:], stdb)
            meant = const.tile([P, 4], F32, tag="meant")
            nc.sync.dma_start(meant[:], meanb)
            mlt = const.tile([P, 2], F32, tag="mlt")
            nc.sync.dma_start(mlt[:], mlb)
            islt = const.tile([P, 2], F32, tag="islt")
            nc.sync.dma_start(islt[:], islb)

            # ---- softmax(wt) -> wv, broadcast to all partitions ----
            wtt = const.tile([1, 10], F32, tag="wtt")
            nc.sync.dma_start(wtt[:], wtv[None, :])
            mx = const.tile([1, 1], F32, tag="mx")
            nc.vector.reduce_max(mx[:, 0:1], wtt[:], axis=AX.X)
            nm = const.tile([1, 1], F32, tag="nm")
            nc.vector.tensor_scalar_mul(nm[:], mx[:], -1.0)
            ex = const.tile([1, 10], F32, tag="ex")
            nc.scalar.activation(ex[:], wtt[:], AF.Exp, bias=nm[:])
            sm = const.tile([1, 1], F32, tag="sm")
            nc.vector.reduce_sum(sm[:, 0:1], ex[:], axis=AX.X)
            inv = const.tile([1, 1], F32, tag="inv")
            nc.vector.reciprocal(inv[:], sm[:])
            wv10 = const.tile([1, 10], F32, tag="wv10")
            nc.vector.tensor_scalar_mul(wv10[:], ex[:], inv[:])
            # interleave [w0,w0,w1,w1,...] into a zero-padded [32, 32] tile
            wvp = const.tile([32, 32], F32, tag="wvp")
            nc.vector.memset(wvp[:], 0.0)
            nc.vector.tensor_copy(
                wvp[0:1, 0:20].rearrange("p (h c) -> p h c", c=2),
                wv10[:, :, None].to_broadcast([1, 10, 2]),
            )
            onesp = const.tile([32, P], F32, tag="onesp")
            nc.vector.memset(onesp[:], 0.0)
            nc.vector.memset(onesp[0:1, :], 1.0)
            pwv = pstr.tile([P, P], F32, tag="tr")
            nc.tensor.matmul(pwv[:, :32], onesp[:], wvp[:], start=True, stop=True)
            wv20 = const.tile([P, 20], F32, tag="wv20")
            nc.vector.tensor_copy(wv20[:], pwv[:, :20])

            # ---- L1: h1 = relu(W1 @ x^T + b1), K=4 zero-padded to 128 ----
            xtb = const.tile([P, B], BF16, tag="xtb")
            nc.vector.memset(xtb[:], 0.0)
            nc.sync.dma_start(xtb[:4, :], xt)
            w1tb = const.tile([P, 2048], BF16, tag="w1tb")
            nc.vector.memset(w1tb[:], 0.0)
            nc.sync.dma_start(w1tb[:4, :], w1)

            h1 = hpool.tile([P, 16, B], BF16, tag="act")
            for m in range(16):
                for n in range(NB):
                    ps = psum.tile([P, NF], F32, tag="mm")
                    nc.tensor.matmul(
                        ps[:],
                        w1tb[:, m * P : (m + 1) * P],
                        xtb[:, n * NF : (n + 1) * NF],
                        start=True,
                        stop=True,
                    )
                    nc.scalar.activation(
                        h1[:, m, n * NF : (n + 1) * NF], ps[:], AF.Relu,
                        bias=b1t[:, m : m + 1],
                    )

            # ---- generic streamed GEMM layer ----
            def mlp_layer(wdram, KT, MT, MD, hin, kin_base, btile, func, evict):
                for m in range(MT):
                    mp = min(P, MD - m * P)
                    wcol = wpool.tile([P, KT, mp], BF16, tag="wcol")
                    if len(wdram.shape) == 4:
                        nc.sync.dma_start(wcol[:], wdram[:, m])
                    else:
                        nc.sync.dma_start(wcol[:], wdram)
                    for n in range(NB):
                        ps = psum.tile([P, NF], F32, tag="mm")
                        for k in range(KT):
                            nc.tensor.matmul(
                                ps[:mp],
                                wcol[:, k, :],
                                hin[:, kin_base + k, n * NF : (n + 1) * NF],
                                start=(k == 0),
                                stop=(k == KT - 1),
                            )
                        bias = btile[:mp, m : m + 1] if mp == P else btile[:mp]
                        evict(m, n, ps[:mp], bias)

            # ---- L2: h2 = relu(W2 @ h1 + b2) ----
            h2 = hpool.tile([P, 16, B], BF16, tag="act")

            def ev_h(hout):
                def _e(m, n, ps, bias):
                    nc.scalar.activation(
                        hout[:, m, n * NF : (n + 1) * NF], ps, AF.Relu, bias=bias
                    )
                return _e

            mlp_layer(w2, 16, 16, 2048, h1, 0, b2t, AF.Relu, ev_h(h2))

            # ---- L3: h3 = relu([W31;W32] @ h2 + [b31;b32]) ----
            h3 = hpool.tile([P, 16, B], BF16, tag="act")
            mlp_layer(w3, 16, 16, 2048, h2, 0, b3t, AF.Relu, ev_h(h3))

            # ---- L4: h41 = relu(W41 @ h31), h42 = relu(W42 @ h32) ----
            h4 = hpool.tile([P, 16, B], BF16, tag="act")

            def ev_h4(m_off):
                def _e(m, n, ps, bias):
                    nc.scalar.activation(
                        h4[:, m_off + m, n * NF : (n + 1) * NF], ps, AF.Relu,
                        bias=bias,
                    )
                return _e

            mlp_layer(w41, 8, 8, 1024, h3, 0, b41t, AF.Relu, ev_h4(0))
            mlp_layer(w42, 8, 8, 1024, h3, 8, b42t, AF.Relu, ev_h4(8))

            # ---- L5: x51 = W51 @ h41 + b51 ; x52 = 4*sigmoid(W52 @ h42 + b52)
            x51s = tp.tile([32, B], F32, tag="x51s")
            nc.vector.memset(x51s[:], 0.0)
            x52s = tp.tile([32, B], F32, tag="x52s")
            nc.vector.memset(x52s[:], 0.0)

            def ev_51(m, n, ps, bias):
                nc.scalar.activation(
                    x51s[:20, n * NF : (n + 1) * NF], ps, AF.Identity, bias=bias
                )

            def ev_52(m, n, ps, bias):
                nc.scalar.activation(
                    x52s[:11, n * NF : (n + 1) * NF], ps, AF.Sigmoid, bias=bias
                )

            mlp_layer(w51, 8, 1, 20, h4, 0, b51t, AF.Identity, ev_51)
            mlp_layer(w52, 8, 1, 11, h4, 8, b52t, AF.Sigmoid, ev_52)
            nc.vector.tensor_scalar_mul(x52s[:11, :], x52s[:11, :], 4.0)

            # ---- transpose heads to [p, chunk, feat] via PE transpose ----
            x51t = tp.tile([P, CH, 20], F32, tag="x51t")
            x52t = tp.tile([P, CH, 11], F32, tag="x52t")
            for c in range(CH):
                pt = pstr.tile([P, P], F32, tag="tr")
                nc.tensor.transpose(
                    pt[:, :32], x51s[:, c * P : (c + 1) * P], ident[:32, :32]
                )
                nc.vector.tensor_copy(x51t[:, c, :], pt[:, :20])
                pt2 = pstr.tile([P, P], F32, tag="tr")
                nc.tensor.transpose(
                    pt2[:, :32], x52s[:, c * P : (c + 1) * P], ident[:32, :32]
                )
                nc.vector.tensor_copy(x52t[:, c, :], pt2[:, :11])

            # ---- tail: trig, barrier, per-head analytic QP ----
            def t3(tag, d=1):
                return tp.tile([P, CH, d], F32, tag=tag, name=tag)

            xnt = t3("xnt", 4)
            nc.sync.dma_start(xnt[:], xn)
            x0 = t3("x0", 4)
            nc.vector.tensor_mul(
                x0[:], xnt[:], stdt[:, None, :].to_broadcast([P, CH, 4])
            )
            nc.vector.tensor_add(
                x0[:], x0[:], meant[:, None, :].to_broadcast([P, CH, 4])
            )

            th = x0[:, :, 0::2]   # [P, CH, 2] angles
            wv_ = x0[:, :, 1::2]  # [P, CH, 2] angular velocities

            # range-reduce th -> rs in [-pi, pi]:  rs = th - 2pi*rint(th/2pi)
            q = t3("q", 2)
            qi = tp.tile([P, CH, 2], I32, tag="qi")
            qr = t3("qr", 2)
            rs = t3("rs", 2)
            nc.vector.tensor_scalar_mul(q[:], th, 1.0 / TWO_PI)
            nc.vector.tensor_copy(qi[:], q[:])
            nc.vector.tensor_copy(qr[:], qi[:])
            nc.vector.scalar_tensor_tensor(
                rs[:], in0=qr[:], scalar=-TWO_PI, in1=th,
                op0=ALU.mult, op1=ALU.add,
            )
            # range-reduce th + pi/2 -> rc (for cos)
            qc = t3("qc", 2)
            qci = tp.tile([P, CH, 2], I32, tag="qci")
            qcr = t3("qcr", 2)
            rc = t3("rc", 2)
            nc.vector.tensor_scalar(
                qc[:], th, 1.0 / TWO_PI, 0.25, op0=ALU.mult, op1=ALU.add
            )
            nc.vector.tensor_copy(qci[:], qc[:])
            nc.vector.tensor_copy(qcr[:], qci[:])
            nc.vector.scalar_tensor_tensor(
                rc[:], in0=qcr[:], scalar=-TWO_PI, in1=th,
                op0=ALU.mult, op1=ALU.add,
            )
            nc.vector.tensor_scalar_add(rc[:], rc[:], HALF_PI)

            sn = t3("sn", 2)
            cs = t3("cs", 2)
            nc.scalar.activation(sn[:], rs[:], AF.Sin)
            nc.scalar.activation(cs[:], rc[:], AF.Sin)

            s1, s2 = sn[:, :, 0:1], sn[:, :, 1:2]
            c1, c2 = cs[:, :, 0:1], cs[:, :, 1:2]
            w1v, w2v = wv_[:, :, 0:1], wv_[:, :, 1:2]

            px = t3("px")
            nc.vector.tensor_add(px[:], c1, c2)
            nc.vector.tensor_scalar_mul(px[:], px[:], 3.0)
            py = t3("py")
            nc.vector.tensor_add(py[:], s1, s2)
            nc.vector.tensor_scalar(py[:], py[:], 3.0, -7.0, op0=ALU.mult, op1=ALU.add)

            s1w = t3("s1w")
            nc.vector.tensor_mul(s1w[:], s1, w1v)
            s2w = t3("s2w")
            nc.vector.tensor_mul(s2w[:], s2, w2v)
            vx = t3("vx")
            nc.vector.tensor_add(vx[:], s1w[:], s2w[:])
            nc.vector.tensor_scalar_mul(vx[:], vx[:], -3.0)
            c1w = t3("c1w")
            nc.vector.tensor_mul(c1w[:], c1, w1v)
            c2w = t3("c2w")
            nc.vector.tensor_mul(c2w[:], c2, w2v)
            vy = t3("vy")
            nc.vector.tensor_add(vy[:], c1w[:], c2w[:])
            nc.vector.tensor_scalar_mul(vy[:], vy[:], 3.0)

            pxx = t3("pxx")
            nc.vector.tensor_mul(pxx[:], px[:], px[:])
            pyy = t3("pyy")
            nc.vector.tensor_mul(pyy[:], py[:], py[:])
            barrier = t3("barrier")
            nc.vector.tensor_add(barrier[:], pxx[:], pyy[:])
            nc.vector.tensor_scalar_add(barrier[:], barrier[:], -16.0)

            pv1 = t3("pv1")
            nc.vector.tensor_mul(pv1[:], px[:], vx[:])
            pv2 = t3("pv2")
            nc.vector.tensor_mul(pv2[:], py[:], vy[:])
            b_dot = t3("b_dot")
            nc.vector.tensor_add(b_dot[:], pv1[:], pv2[:])
            nc.vector.tensor_scalar_mul(b_dot[:], b_dot[:], 2.0)

            w1sq = t3("w1sq")
            nc.vector.tensor_mul(w1sq[:], w1v, w1v)
            w2sq = t3("w2sq")
            nc.vector.tensor_mul(w2sq[:], w2v, w2v)
            ca = t3("ca")
            nc.vector.tensor_mul(ca[:], c1, w1sq[:])
            cb = t3("cb")
            nc.vector.tensor_mul(cb[:], c2, w2sq[:])
            nc.vector.tensor_add(ca[:], ca[:], cb[:])   # c1*w1^2 + c2*w2^2
            sa = t3("sa")
            nc.vector.tensor_mul(sa[:], s1, w1sq[:])
            sb = t3("sb")
            nc.vector.tensor_mul(sb[:], s2, w2sq[:])
            nc.vector.tensor_add(sa[:], sa[:], sb[:])   # s1*w1^2 + s2*w2^2

            vxx = t3("vxx")
            nc.vector.tensor_mul(vxx[:], vx[:], vx[:])
            vyy = t3("vyy")
            nc.vector.tensor_mul(vyy[:], vy[:], vy[:])
            vsum = t3("vsum")
            nc.vector.tensor_add(vsum[:], vxx[:], vyy[:])
            nc.vector.tensor_scalar_mul(vsum[:], vsum[:], 2.0)  # 2vx^2+2vy^2

            pca = t3("pca")
            nc.vector.tensor_mul(pca[:], px[:], ca[:])
            psa = t3("psa")
            nc.vector.tensor_mul(psa[:], py[:], sa[:])
            nc.vector.tensor_add(pca[:], pca[:], psa[:])
            lf2b = t3("lf2b")
            nc.vector.scalar_tensor_tensor(
                lf2b[:], in0=pca[:], scalar=-6.0, in1=vsum[:],
                op0=ALU.mult, op1=ALU.add,
            )  # Lf2b = 2(vx^2+vy^2) - 6*(px*ca + py*sa)

            g1 = t3("g1")
            m1 = t3("m1")
            nc.vector.tensor_mul(m1[:], py[:], c1)
            m2 = t3("m2")
            nc.vector.tensor_mul(m2[:], px[:], s1)
            nc.vector.tensor_sub(g1[:], m1[:], m2[:])
            nc.vector.tensor_scalar_mul(g1[:], g1[:], 6.0)
            g2 = t3("g2")
            nc.vector.tensor_mul(m1[:], py[:], c2)
            nc.vector.tensor_mul(m2[:], px[:], s2)
            nc.vector.tensor_sub(g2[:], m1[:], m2[:])
            nc.vector.tensor_scalar_mul(g2[:], g2[:], 6.0)

            gdot = t3("gdot")
            g1sq = t3("g1sq")
            nc.vector.tensor_mul(g1sq[:], g1[:], g1[:])
            g2sq = t3("g2sq")
            nc.vector.tensor_mul(g2sq[:], g2[:], g2[:])
            nc.vector.tensor_add(gdot[:], g1sq[:], g2sq[:])
            igdot = t3("igdot")
            nc.vector.reciprocal(igdot[:], gdot[:])

            # per-head: hval, viol, lam
            B10 = [P, CH, 10]
            alpha = x52t[:, :, 0:1]
            betas = x52t[:, :, 1:11]
            apb = t3("apb", 10)
            nc.vector.tensor_add(apb[:], betas, alpha.to_broadcast(B10))
            ab = t3("ab", 10)
            nc.vector.tensor_mul(ab[:], betas, alpha.to_broadcast(B10))
            hv = t3("hv", 10)
            nc.vector.tensor_mul(hv[:], apb[:], b_dot[:].to_broadcast(B10))
            hv2 = t3("hv2", 10)
            nc.vector.tensor_mul(hv2[:], ab[:], barrier[:].to_broadcast(B10))
            nc.vector.tensor_add(hv[:], hv[:], hv2[:])
            nc.vector.tensor_add(hv[:], hv[:], lf2b[:].to_broadcast(B10))

            refx = x51t[:, :, 0::2]
            refy = x51t[:, :, 1::2]
            viol = t3("viol", 10)
            v2t = t3("v2t", 10)
            nc.vector.tensor_mul(viol[:], refx, g1[:].to_broadcast(B10))
            nc.vector.tensor_mul(v2t[:], refy, g2[:].to_broadcast(B10))
            nc.vector.tensor_add(viol[:], viol[:], v2t[:])
            nc.vector.tensor_sub(viol[:], viol[:], hv[:])

            lam = t3("lam", 10)
            nc.vector.tensor_mul(lam[:], viol[:], igdot[:].to_broadcast(B10))
            nc.vector.tensor_scalar_max(lam[:], lam[:], 0.0)

            # S = sum_h wv_h * lam_h ; refbar = sum_h wv_h * ref_h
            wlam = t3("wlam", 10)
            nc.vector.tensor_mul(
                wlam[:], lam[:], wv20[:, None, 0::2].to_broadcast(B10)
            )
            S = t3("S")
            nc.vector.reduce_sum(S[:, :, 0], wlam[:], axis=AX.X)

            wref = t3("wref", 20)
            nc.vector.tensor_mul(
                wref[:], x51t[:], wv20[:, None, :].to_broadcast([P, CH, 20])
            )
            rbx = t3("rbx")
            nc.vector.reduce_sum(rbx[:, :, 0], wref[:, :, 0::2], axis=AX.X)
            rby = t3("rby")
            nc.vector.reduce_sum(rby[:, :, 0], wref[:, :, 1::2], axis=AX.X)

            rtx = t3("rtx")
            nc.vector.tensor_mul(rtx[:], S[:], g1[:])
            nc.vector.tensor_sub(rtx[:], rtx[:], rbx[:])
            rty = t3("rty")
            nc.vector.tensor_mul(rty[:], S[:], g2[:])
            nc.vector.tensor_sub(rty[:], rty[:], rby[:])

            ot = t3("ot", 2)
            nc.vector.tensor_scalar(
                ot[:, :, 0], rtx[:, :, 0], mlt[:, 0:1], islt[:, 0:1],
                op0=ALU.subtract, op1=ALU.mult,
            )
            nc.vector.tensor_scalar(
                ot[:, :, 1], rty[:, :, 0], mlt[:, 1:2], islt[:, 1:2],
                op0=ALU.subtract, op1=ALU.mult,
            )
            nc.sync.dma_start(out, ot[:])

    nc.compile()
    return nc


def _get_nc():
    global _CACHED_NC
    if _CACHED_NC is None:
        _CACHED_NC = _build()
    return _CACHED_NC


def _bf16(a):
    return np.ascontiguousarray(a.astype(ml_dtypes.bfloat16))


def _f32(a):
    return np.ascontiguousarray(np.asarray(a, dtype=np.float32))


def _prep_inputs(inputs):
    x = _f32(inputs["x"])
    mean = _f32(inputs["mean"])
    std = _f32(inputs["std"])
    mean_label = _f32(inputs["mean_label"])
    std_label = _f32(inputs["std_label"])
    wt = _f32(inputs["wt"])
    W1, b1 = _f32(inputs["W1"]), _f32(inputs["b1"])
    W2, b2 = _f32(inputs["W2"]), _f32(inputs["b2"])
    W31, b31 = _f32(inputs["W31"]), _f32(inputs["b31"])
    W32, b32 = _f32(inputs["W32"]), _f32(inputs["b32"])
    W41, b41 = _f32(inputs["W41"]), _f32(inputs["b41"])
    W42, b42 = _f32(inputs["W42"]), _f32(inputs["b42"])
    W51, b51 = _f32(inputs["W51"]), _f32(inputs["b51"])
    W52, b52 = _f32(inputs["W52"]), _f32(inputs["b52"])

    def pack4(wT, KT, MT):  # (K, M) -> (128, MT, KT, 128)
        K, M = wT.shape
        return _bf16(
            wT.reshape(KT, P, MT, P).transpose(1, 2, 0, 3)
        )

    def pack3(wT, KT):  # (K, M) -> (128, KT, M)
        K, M = wT.shape
        return _bf16(wT.reshape(KT, P, M).transpose(1, 0, 2))

    W3T = np.concatenate([W31.T, W32.T], axis=1)  # (2048, 2048)
    b3 = np.concatenate([b31, b32])

    shared = {
        "w1": _bf16(W1.T),
        "w2": pack4(W2.T, 16, 16),
        "w3": pack4(W3T, 16, 16),
        "w41": pack4(W41.T, 8, 8),
        "w42": pack4(W42.T, 8, 8),
        "w51": pack3(W51.T, 8),
        "w52": pack3(W52.T, 8),
        "b1": _f32(b1.reshape(16, P).T),
        "b2": _f32(b2.reshape(16, P).T),
        "b3": _f32(b3.reshape(16, P).T),
        "b41": _f32(b41.reshape(8, P).T),
        "b42": _f32(b42.reshape(8, P).T),
        "b51": b51,
        "b52": b52,
        "stdb": _f32(np.tile(std[None, :], (P, 1))),
        "meanb": _f32(np.tile(mean[None, :], (P, 1))),
        "mlb": _f32(np.tile(mean_label[None, :], (P, 1))),
        "islb": _f32(np.tile((1.0 / std_label)[None, :], (P, 1))),
        "wtv": wt,
    }

    in_maps = []
    for i in range(N_CORES):
        xs = x[i * B : (i + 1) * B]  # (2048, 4)
        m = dict(shared)
        m["xt"] = _bf16(xs.T)
        m["xn"] = _f32(xs.reshape(CH, P, 4).transpose(1, 0, 2))
        in_maps.append(m)
    return in_maps


def kernel_core(inputs, trace=False):
    nc = _get_nc()
    in_maps = _prep_inputs(inputs)
    res = run_bass_kernel_spmd(
        nc, in_maps, core_ids=list(range(N_CORES)), trace=trace
    )
    shards = []
    for i in range(N_CORES):
        o = res.results[i]["out"]  # (128, 16, 2)
        shards.append(o.transpose(1, 0, 2).reshape(B, 2))
    return np.concatenate(shards, axis=0).astype(np.float32), res


def kernel(**inputs):
    out, _ = kernel_core(inputs)
    return out


# revision 4
# speedup vs baseline: 1.0366x; 1.0366x over previous
"""Trainium2 Bass kernel for nn_ABNet_U (multi-branch MLP + CBF-QP head).

Data-parallel over batch: 16384 rows -> 8 NeuronCores x 2048 rows.
Weights replicated, host-prepped into K-major bf16 layouts; all GEMMs run
on the TensorEngine with fp32 PSUM accumulation and fused bias+activation
eviction on the Scalar/Vector engines.  The trig/QP tail runs in fp32 on
the Vector/Scalar engines with batch on partitions, emitted early so it
overlaps the GEMM phase.
"""

import sys

sys.path.insert(0, "/opt/trn_rl_repo")

import numpy as np
import ml_dtypes

import concourse.bass as bass
import concourse.mybir as mybir
import concourse.tile as tile
from concourse import bacc
from concourse.bass_utils import run_bass_kernel_spmd
from concourse.masks import make_identity

N_CORES = 8
B_GLOBAL = 16384
B = B_GLOBAL // N_CORES  # 2048 rows per core
P = 128
CH = B // P              # 16 batch chunks of 128 (tail layout)
NF = 512                 # matmul free-dim chunk
NB = B // NF             # 4 free chunks
HEADS = 10

AF = mybir.ActivationFunctionType
ALU = mybir.AluOpType
AX = mybir.AxisListType
F32 = mybir.dt.float32
BF16 = mybir.dt.bfloat16
I32 = mybir.dt.int32

TWO_PI = float(2.0 * np.pi)
HALF_PI = float(0.5 * np.pi)

_CACHED_NC = None


def _build():
    nc = bacc.Bacc(
        "TRN2",
        target_bir_lowering=False,
        debug=False,
        enable_asserts=False,
        num_devices=N_CORES,
    )

    def din(name, shape, dt=F32):
        return nc.dram_tensor(name, list(shape), dt, kind="ExternalInput").ap()

    xt = din("xt", (4, B), BF16)            # x shard, transposed, bf16
    xn = din("xn", (P, CH, 4))              # x shard, [p, chunk, feat] fp32
    w1 = din("w1", (4, 2048), BF16)
    w2 = din("w2", (P, 16, 16, P), BF16)    # [p, mt, kt, mc]
    w3 = din("w3", (P, 16, 16, P), BF16)
    w41 = din("w41", (P, 8, 8, P), BF16)
    w42 = din("w42", (P, 8, 8, P), BF16)
    w51 = din("w51", (P, 8, 20), BF16)      # [p, kt, m]
    w52 = din("w52", (P, 8, 11), BF16)
    b1 = din("b1", (P, 16))
    b2 = din("b2", (P, 16))
    b3 = din("b3", (P, 16))
    b41 = din("b41", (P, 8))
    b42 = din("b42", (P, 8))
    b51 = din("b51", (20,))
    b52 = din("b52", (11,))
    stdb = din("stdb", (P, 4))
    meanb = din("meanb", (P, 4))
    mlb = din("mlb", (P, 2))
    islb = din("islb", (P, 2))
    wtv = din("wtv", (10,))
    out = nc.dram_tensor("out", [P, CH, 2], F32, kind="ExternalOutput").ap()

    with tile.TileContext(nc) as tc:
        from contextlib import ExitStack

        with ExitStack() as ctx:
            const = ctx.enter_context(tc.tile_pool(name="const", bufs=1))
            wpool = ctx.enter_context(tc.tile_pool(name="wpool", bufs=3))
            hpool = ctx.enter_context(tc.tile_pool(name="hpool", bufs=2))
            psum = ctx.enter_context(tc.tile_pool(name="psum", bufs=6, space="PSUM"))
            pstr = ctx.enter_context(tc.tile_pool(name="pstr", bufs=2, space="PSUM"))
            tp = ctx.enter_context(tc.tile_pool(name="tp", bufs=1))

            # ---- L1-critical loads first: keep the PE fed from t=0 ----
            b1t = const.tile([P, 16], F32, tag="b1")
            nc.sync.dma_start(b1t[:], b1)
            xtb = const.tile([P, B], BF16, tag="xtb")
            nc.vector.memset(xtb[:], 0.0)
            nc.sync.dma_start(xtb[:4, :], xt)
            w1tb = const.tile([P, 2048], BF16, tag="w1tb")
            nc.vector.memset(w1tb[:], 0.0)
            nc.sync.dma_start(w1tb[:4, :], w1)

            # ---- L1: h1 = relu(W1 @ x^T + b1), K=4 zero-padded to 128 ----
            # One matmul per eviction, so evictions bound this layer: split
            # them across the Scalar and Vector engines.
            h1 = hpool.tile([P, 16, B], BF16, tag="act")
            flip = 0
            for m in range(16):
                for n in range(NB):
                    ps = psum.tile([P, NF], F32, tag="mm")
                    nc.tensor.matmul(
                        ps[:],
                        w1tb[:, m * P : (m + 1) * P],
                        xtb[:, n * NF : (n + 1) * NF],
                        start=True,
                        stop=True,
                    )
                    dst = h1[:, m, n * NF : (n + 1) * NF]
                    if flip % 2 == 0:
                        nc.scalar.activation(
                            dst, ps[:], AF.Relu, bias=b1t[:, m : m + 1]
                        )
                    else:
                        nc.vector.tensor_scalar(
                            dst, ps[:], b1t[:, m : m + 1], 0.0,
                            op0=ALU.add, op1=ALU.max,
                        )
                    flip += 1

            # ---- remaining constants (emitted after L1 so they never gate it)
            b2t = const.tile([P, 16], F32, tag="b2")
            nc.sync.dma_start(b2t[:], b2)
            b3t = const.tile([P, 16], F32, tag="b3")
            nc.sync.dma_start(b3t[:], b3)
            b41t = const.tile([P, 8], F32, tag="b41")
            nc.sync.dma_start(b41t[:], b41)
            b42t = const.tile([P, 8], F32, tag="b42")
            nc.sync.dma_start(b42t[:], b42)
            b51t = const.tile([20, 1], F32, tag="b51")
            nc.sync.dma_start(b51t[:], b51[:, None])
            b52t = const.tile([11, 1], F32, tag="b52")
            nc.sync.dma_start(b52t[:], b52[:, None])
            stdt = const.tile([P, 4], F32, tag="stdt")
            nc.sync.dma_start(stdt[:], stdb)
            meant = const.tile([P, 4], F32, tag="meant")
            nc.sync.dma_start(meant[:], meanb)
            mlt = const.tile([P, 2], F32, tag="mlt")
            nc.sync.dma_start(mlt[:], mlb)
            islt = const.tile([P, 2], F32, tag="islt")
            nc.sync.dma_start(islt[:], islb)
            halfpi = const.tile([P, 1], F32, tag="halfpi")
            nc.vector.memset(halfpi[:], HALF_PI)
            ident = const.tile([P, P], F32)
            make_identity(nc, ident[:])

            # softmax(wt) DVE chain (PE broadcast deferred until after L4)
            wtt = const.tile([1, 10], F32, tag="wtt")
            nc.sync.dma_start(wtt[:], wtv[None, :])
            mx = const.tile([1, 1], F32, tag="mx")
            nc.vector.reduce_max(mx[:, 0:1], wtt[:], axis=AX.X)
            nm = const.tile([1, 1], F32, tag="nm")
            nc.vector.tensor_scalar_mul(nm[:], mx[:], -1.0)
            ex = const.tile([1, 10], F32, tag="ex")
            nc.scalar.activation(ex[:], wtt[:], AF.Exp, bias=nm[:])
            sm = const.tile([1, 1], F32, tag="sm")
            nc.vector.reduce_sum(sm[:, 0:1], ex[:], axis=AX.X)
            inv = const.tile([1, 1], F32, tag="inv")
            nc.vector.reciprocal(inv[:], sm[:])
            wv10 = const.tile([1, 10], F32, tag="wv10")
            nc.vector.tensor_scalar_mul(wv10[:], ex[:], inv[:])
            wvp = const.tile([32, 32], F32, tag="wvp")
            nc.vector.memset(wvp[:], 0.0)
            nc.vector.tensor_copy(
                wvp[0:1, 0:20].rearrange("p (h c) -> p h c", c=2),
                wv10[:, :, None].to_broadcast([1, 10, 2]),
            )
            onesp = const.tile([32, P], F32, tag="onesp")
            nc.vector.memset(onesp[:], 0.0)
            nc.vector.memset(onesp[0:1, :], 1.0)

            # combined x51/x52 head tile (rows 0..19 = x51, 20..30 = x52)
            x5cat = tp.tile([64, B], F32, tag="x5cat")
            nc.vector.memset(x5cat[:], 0.0)

            # ---- tail part 1: geometry from x only — emitted now so the
            # Vector engine computes it underneath the L2..L5 GEMMs.
            def t3(tag, d=1):
                return tp.tile([P, CH, d], F32, tag=tag, name=tag)

            xnt = t3("xnt", 4)
            nc.sync.dma_start(xnt[:], xn)
            x0 = t3("x0", 4)
            nc.vector.tensor_mul(
                x0[:], xnt[:], stdt[:, None, :].to_broadcast([P, CH, 4])
            )
            nc.vector.tensor_add(
                x0[:], x0[:], meant[:, None, :].to_broadcast([P, CH, 4])
            )

            th = x0[:, :, 0::2]   # [P, CH, 2] angles
            wv_ = x0[:, :, 1::2]  # [P, CH, 2] angular velocities

            # range-reduce th -> rs in [-pi, pi]:  rs = th - 2pi*rint(th/2pi)
            q = t3("q", 2)
            qi = tp.tile([P, CH, 2], I32, tag="qi")
            qr = t3("qr", 2)
            rs = t3("rs", 2)
            nc.vector.tensor_scalar_mul(q[:], th, 1.0 / TWO_PI)
            nc.vector.tensor_copy(qi[:], q[:])
            nc.vector.tensor_copy(qr[:], qi[:])
            nc.vector.scalar_tensor_tensor(
                rs[:], in0=qr[:], scalar=-TWO_PI, in1=th,
                op0=ALU.mult, op1=ALU.add,
            )
            # range-reduce th + pi/2 -> rc (for cos)
            qc = t3("qc", 2)
            qci = tp.tile([P, CH, 2], I32, tag="qci")
            qcr = t3("qcr", 2)
            rc = t3("rc", 2)
            nc.vector.tensor_scalar(
                qc[:], th, 1.0 / TWO_PI, 0.25, op0=ALU.mult, op1=ALU.add
            )
            nc.vector.tensor_copy(qci[:], qc[:])
            nc.vector.tensor_copy(qcr[:], qci[:])
            nc.vector.scalar_tensor_tensor(
                rc[:], in0=qcr[:], scalar=-TWO_PI, in1=th,
                op0=ALU.mult, op1=ALU.add,
            )
            nc.vector.tensor_scalar_add(rc[:], rc[:], HALF_PI)

            sn = t3("sn", 2)
            cs = t3("cs", 2)
            nc.scalar.activation(sn[:], rs[:], AF.Sin)
            nc.scalar.activation(cs[:], rc[:], AF.Sin)

            s1, s2 = sn[:, :, 0:1], sn[:, :, 1:2]
            c1, c2 = cs[:, :, 0:1], cs[:, :, 1:2]
            w1v, w2v = wv_[:, :, 0:1], wv_[:, :, 1:2]

            px = t3("px")
            nc.vector.tensor_add(px[:], c1, c2)
            nc.vector.tensor_scalar_mul(px[:], px[:], 3.0)
            py = t3("py")
            nc.vector.tensor_add(py[:], s1, s2)
            nc.vector.tensor_scalar(py[:], py[:], 3.0, -7.0, op0=ALU.mult, op1=ALU.add)

            s1w = t3("s1w")
            nc.vector.tensor_mul(s1w[:], s1, w1v)
            s2w = t3("s2w")
            nc.vector.tensor_mul(s2w[:], s2, w2v)
            vx = t3("vx")
            nc.vector.tensor_add(vx[:], s1w[:], s2w[:])
            nc.vector.tensor_scalar_mul(vx[:], vx[:], -3.0)
            c1w = t3("c1w")
            nc.vector.tensor_mul(c1w[:], c1, w1v)
            c2w = t3("c2w")
            nc.vector.tensor_mul(c2w[:], c2, w2v)
            vy = t3("vy")
            nc.vector.tensor_add(vy[:], c1w[:], c2w[:])
            nc.vector.tensor_scalar_mul(vy[:], vy[:], 3.0)

            pxx = t3("pxx")
            nc.vector.tensor_mul(pxx[:], px[:], px[:])
            pyy = t3("pyy")
            nc.vector.tensor_mul(pyy[:], py[:], py[:])
            barrier = t3("barrier")
            nc.vector.tensor_add(barrier[:], pxx[:], pyy[:])
            nc.vector.tensor_scalar_add(barrier[:], barrier[:], -16.0)

            pv1 = t3("pv1")
            nc.vector.tensor_mul(pv1[:], px[:], vx[:])
            pv2 = t3("pv2")
            nc.vector.tensor_mul(pv2[:], py[:], vy[:])
            b_dot = t3("b_dot")
            nc.vector.tensor_add(b_dot[:], pv1[:], pv2[:])
            nc.vector.tensor_scalar_mul(b_dot[:], b_dot[:], 2.0)

            w1sq = t3("w1sq")
            nc.vector.tensor_mul(w1sq[:], w1v, w1v)
            w2sq = t3("w2sq")
            nc.vector.tensor_mul(w2sq[:], w2v, w2v)
            ca = t3("ca")
            nc.vector.tensor_mul(ca[:], c1, w1sq[:])
            cb = t3("cb")
            nc.vector.tensor_mul(cb[:], c2, w2sq[:])
            nc.vector.tensor_add(ca[:], ca[:], cb[:])   # c1*w1^2 + c2*w2^2
            sa = t3("sa")
            nc.vector.tensor_mul(sa[:], s1, w1sq[:])
            sb = t3("sb")
            nc.vector.tensor_mul(sb[:], s2, w2sq[:])
            nc.vector.tensor_add(sa[:], sa[:], sb[:])   # s1*w1^2 + s2*w2^2

            vxx = t3("vxx")
            nc.vector.tensor_mul(vxx[:], vx[:], vx[:])
            vyy = t3("vyy")
            nc.vector.tensor_mul(vyy[:], vy[:], vy[:])
            vsum = t3("vsum")
            nc.vector.tensor_add(vsum[:], vxx[:], vyy[:])
            nc.vector.tensor_scalar_mul(vsum[:], vsum[:], 2.0)  # 2vx^2+2vy^2

            pca = t3("pca")
            nc.vector.tensor_mul(pca[:], px[:], ca[:])
            psa = t3("psa")
            nc.vector.tensor_mul(psa[:], py[:], sa[:])
            nc.vector.tensor_add(pca[:], pca[:], psa[:])
            lf2b = t3("lf2b")
            nc.vector.scalar_tensor_tensor(
                lf2b[:], in0=pca[:], scalar=-6.0, in1=vsum[:],
                op0=ALU.mult, op1=ALU.add,
            )  # Lf2b = 2(vx^2+vy^2) - 6*(px*ca + py*sa)

            g1 = t3("g1")
            m1 = t3("m1")
            nc.vector.tensor_mul(m1[:], py[:], c1)
            m2 = t3("m2")
            nc.vector.tensor_mul(m2[:], px[:], s1)
            nc.vector.tensor_sub(g1[:], m1[:], m2[:])
            nc.vector.tensor_scalar_mul(g1[:], g1[:], 6.0)
            g2 = t3("g2")
            nc.vector.tensor_mul(m1[:], py[:], c2)
            nc.vector.tensor_mul(m2[:], px[:], s2)
            nc.vector.tensor_sub(g2[:], m1[:], m2[:])
            nc.vector.tensor_scalar_mul(g2[:], g2[:], 6.0)

            gdot = t3("gdot")
            g1sq = t3("g1sq")
            nc.vector.tensor_mul(g1sq[:], g1[:], g1[:])
            g2sq = t3("g2sq")
            nc.vector.tensor_mul(g2sq[:], g2[:], g2[:])
            nc.vector.tensor_add(gdot[:], g1sq[:], g2sq[:])
            igdot = t3("igdot")
            nc.vector.reciprocal(igdot[:], gdot[:])

            # ---- generic streamed GEMM layer ----
            def mlp_layer(wdram, KT, MT, MD, hin, kin_base, btile, evict):
                for m in range(MT):
                    mp = min(P, MD - m * P)
                    wcol = wpool.tile([P, KT, mp], BF16, tag="wcol")
                    if len(wdram.shape) == 4:
                        nc.sync.dma_start(wcol[:], wdram[:, m])
                    else:
                        nc.sync.dma_start(wcol[:], wdram)
                    for n in range(NB):
                        ps = psum.tile([P, NF], F32, tag="mm")
                        for k in range(KT):
                            nc.tensor.matmul(
                                ps[:mp],
                                wcol[:, k, :],
                                hin[:, kin_base + k, n * NF : (n + 1) * NF],
                                start=(k == 0),
                                stop=(k == KT - 1),
                            )
                        evict(m, n, ps[:mp])

            # ---- L2 / L3 / L4 ----
            h2 = hpool.tile([P, 16, B], BF16, tag="act")

            def ev_h(hout, btile, m_off=0):
                def _e(m, n, ps):
                    nc.scalar.activation(
                        hout[:, m_off + m, n * NF : (n + 1) * NF], ps, AF.Relu,
                        bias=btile[:, m : m + 1],
                    )
                return _e

            mlp_layer(w2, 16, 16, 2048, h1, 0, b2t, ev_h(h2, b2t))

            h3 = hpool.tile([P, 16, B], BF16, tag="act")
            mlp_layer(w3, 16, 16, 2048, h2, 0, b3t, ev_h(h3, b3t))

            h4 = hpool.tile([P, 16, B], BF16, tag="act")
            mlp_layer(w41, 8, 8, 1024, h3, 0, b41t, ev_h(h4, b41t, 0))
            mlp_layer(w42, 8, 8, 1024, h3, 8, b42t, ev_h(h4, b42t, 8))

            # ---- L5 into the combined head tile ----
            def ev_51(m, n, ps):
                nc.scalar.activation(
                    x5cat[:20, n * NF : (n + 1) * NF], ps, AF.Identity,
                    bias=b51t[:],
                )

            def ev_52(m, n, ps):
                nc.scalar.activation(
                    x5cat[32:43, n * NF : (n + 1) * NF], ps, AF.Sigmoid,
                    bias=b52t[:],
                )

            mlp_layer(w51, 8, 1, 20, h4, 0, b51t, ev_51)
            mlp_layer(w52, 8, 1, 11, h4, 8, b52t, ev_52)
            nc.vector.tensor_scalar_mul(x5cat[32:43, :], x5cat[32:43, :], 4.0)

            # wv broadcast to all partitions (PE hits this after L4/L5)
            pwv = pstr.tile([P, 64], F32, tag="tr")
            nc.tensor.matmul(pwv[:, :32], onesp[:], wvp[:], start=True, stop=True)
            wv20 = const.tile([P, 20], F32, tag="wv20")
            nc.vector.tensor_copy(wv20[:], pwv[:, :20])

            # ---- transpose the combined head tile: 16 chunks of [32,128] ----
            x5t = tp.tile([P, CH, 43], F32, tag="x5t")
            for c in range(CH):
                pt = pstr.tile([P, 64], F32, tag="tr")
                nc.tensor.transpose(
                    pt[:, :64], x5cat[:, c * P : (c + 1) * P], ident[:64, :64]
                )
                if c % 2 == 0:
                    nc.vector.tensor_copy(x5t[:, c, :], pt[:, :43])
                else:
                    nc.scalar.copy(x5t[:, c, :], pt[:, :43])

            # ---- tail part 2: per-head analytic QP ----
            B10 = [P, CH, 10]
            alpha = x5t[:, :, 32:33]
            betas = x5t[:, :, 33:43]
            apb = t3("apb", 10)
            nc.vector.tensor_add(apb[:], betas, alpha.to_broadcast(B10))
            ab = t3("ab", 10)
            nc.vector.tensor_mul(ab[:], betas, alpha.to_broadcast(B10))
            hv = t3("hv", 10)
            nc.vector.tensor_mul(hv[:], apb[:], b_dot[:].to_broadcast(B10))
            hv2 = t3("hv2", 10)
            nc.vector.tensor_mul(hv2[:], ab[:], barrier[:].to_broadcast(B10))
            nc.vector.tensor_add(hv[:], hv[:], hv2[:])
            nc.vector.tensor_add(hv[:], hv[:], lf2b[:].to_broadcast(B10))

            refx = x5t[:, :, 0:20:2]
            refy = x5t[:, :, 1:20:2]
            viol = t3("viol", 10)
            v2t = t3("v2t", 10)
            nc.vector.tensor_mul(viol[:], refx, g1[:].to_broadcast(B10))
            nc.vector.tensor_mul(v2t[:], refy, g2[:].to_broadcast(B10))
            nc.vector.tensor_add(viol[:], viol[:], v2t[:])
            nc.vector.tensor_sub(viol[:], viol[:], hv[:])

            lam = t3("lam", 10)
            nc.vector.tensor_mul(lam[:], viol[:], igdot[:].to_broadcast(B10))
            nc.vector.tensor_scalar_max(lam[:], lam[:], 0.0)

            # S = sum_h wv_h * lam_h ; refbar = sum_h wv_h * ref_h
            wlam = t3("wlam", 10)
            nc.vector.tensor_mul(
                wlam[:], lam[:], wv20[:, None, 0::2].to_broadcast(B10)
            )
            S = t3("S")
            nc.vector.reduce_sum(S[:, :, 0], wlam[:], axis=AX.X)

            wref = t3("wref", 20)
            nc.vector.tensor_mul(
                wref[:], x5t[:, :, 0:20], wv20[:, None, :].to_broadcast([P, CH, 20])
            )
            rbx = t3("rbx")
            nc.vector.reduce_sum(rbx[:, :, 0], wref[:, :, 0::2], axis=AX.X)
            rby = t3("rby")
            nc.vector.reduce_sum(rby[:, :, 0], wref[:, :, 1::2], axis=AX.X)

            rtx = t3("rtx")
            nc.vector.tensor_mul(rtx[:], S[:], g1[:])
            nc.vector.tensor_sub(rtx[:], rtx[:], rbx[:])
            rty = t3("rty")
            nc.vector.tensor_mul(rty[:], S[:], g2[:])
            nc.vector.tensor_sub(rty[:], rty[:], rby[:])

            ot = t3("ot", 2)
            nc.vector.tensor_scalar(
                ot[:, :, 0], rtx[:, :, 0], mlt[:, 0:1], islt[:, 0:1],
                op0=ALU.subtract, op1=ALU.mult,
            )
            nc.vector.tensor_scalar(
                ot[:, :, 1], rty[:, :, 0], mlt[:, 1:2], islt[:, 1:2],
                op0=ALU.subtract, op1=ALU.mult,
            )
            nc.sync.dma_start(out, ot[:])

    nc.compile()
    return nc


def _get_nc():
    global _CACHED_NC
    if _CACHED_NC is None:
        _CACHED_NC = _build()
    return _CACHED_NC


def _bf16(a):
    return np.ascontiguousarray(a.astype(ml_dtypes.bfloat16))


def _f32(a):
    return np.ascontiguousarray(np.asarray(a, dtype=np.float32))


def _prep_inputs(inputs):
    x = _f32(inputs["x"])
    mean = _f32(inputs["mean"])
    std = _f32(inputs["std"])
    mean_label = _f32(inputs["mean_label"])
    std_label = _f32(inputs["std_label"])
    wt = _f32(inputs["wt"])
    W1, b1 = _f32(inputs["W1"]), _f32(inputs["b1"])
    W2, b2 = _f32(inputs["W2"]), _f32(inputs["b2"])
    W31, b31 = _f32(inputs["W31"]), _f32(inputs["b31"])
    W32, b32 = _f32(inputs["W32"]), _f32(inputs["b32"])
    W41, b41 = _f32(inputs["W41"]), _f32(inputs["b41"])
    W42, b42 = _f32(inputs["W42"]), _f32(inputs["b42"])
    W51, b51 = _f32(inputs["W51"]), _f32(inputs["b51"])
    W52, b52 = _f32(inputs["W52"]), _f32(inputs["b52"])

    def pack4(wT, KT, MT):  # (K, M) -> (128, MT, KT, 128)
        return _bf16(wT.reshape(KT, P, MT, P).transpose(1, 2, 0, 3))

    def pack3(wT, KT):  # (K, M) -> (128, KT, M)
        K, M = wT.shape
        return _bf16(wT.reshape(KT, P, M).transpose(1, 0, 2))

    W3T = np.concatenate([W31.T, W32.T], axis=1)  # (2048, 2048)
    b3 = np.concatenate([b31, b32])

    shared = {
        "w1": _bf16(W1.T),
        "w2": pack4(W2.T, 16, 16),
        "w3": pack4(W3T, 16, 16),
        "w41": pack4(W41.T, 8, 8),
        "w42": pack4(W42.T, 8, 8),
        "w51": pack3(W51.T, 8),
        "w52": pack3(W52.T, 8),
        "b1": _f32(b1.reshape(16, P).T),
        "b2": _f32(b2.reshape(16, P).T),
        "b3": _f32(b3.reshape(16, P).T),
        "b41": _f32(b41.reshape(8, P).T),
        "b42": _f32(b42.reshape(8, P).T),
        "b51": b51,
        "b52": b52,
        "stdb": _f32(np.tile(std[None, :], (P, 1))),
        "meanb": _f32(np.tile(mean[None, :], (P, 1))),
        "mlb": _f32(np.tile(mean_label[None, :], (P, 1))),
        "islb": _f32(np.tile((1.0 / std_label)[None, :], (P, 1))),
        "wtv": wt,
    }

    in_maps = []
    for i in range(N_CORES):
        xs = x[i * B : (i + 1) * B]  # (2048, 4)
        m = dict(shared)
        m["xt"] = _bf16(xs.T)
        m["xn"] = _f32(xs.reshape(CH, P, 4).transpose(1, 0, 2))
        in_maps.append(m)
    return in_maps


def kernel_core(inputs, trace=False):
    nc = _get_nc()
    in_maps = _prep_inputs(inputs)
    res = run_bass_kernel_spmd(
        nc, in_maps, core_ids=list(range(N_CORES)), trace=trace
    )
    shards = []
    for i in range(N_CORES):
        o = res.results[i]["out"]  # (128, 16, 2)
        shards.append(o.transpose(1, 0, 2).reshape(B, 2))
    return np.concatenate(shards, axis=0).astype(np.float32), res


def kernel(**inputs):
    out, _ = kernel_core(inputs)
    return out
